# revision 1
# baseline (speedup 1.0000x reference)
"""GNN message passing on 8 Trainium2 NeuronCores.

out = segment_sum(adj_vals * (feat @ W)[adj_cols], adj_rows)
N=100000 nodes, E=1600000 edges, 256 -> 64 features, fp32.

Sharding: nodes (rows of feat / output) across 8 cores; weight replicated;
edges partitioned by destination row (adj_rows sorted -> contiguous); each
core's scatter-add is local, with an AllGather of projected source features.

Device pipeline per core:
  phase 1: proj_local = featT_slice.T @ W       (PE, PSUM-accumulated)
           AllGather(proj_local) -> proj_full   (internal DRAM, Shared)
  phase 2: per batch of G output tiles:
             dma_gather (4 int16-bucket calls) proj rows -> msgs SBUF
             per 128-edge chunk: indicator[128,W] = (iota==rloc)*val  (DVE)
             psumT[64,128] += msgs_chunk.T @ indicator                (PE)
           drain psum -> out[64, :] -> DRAM
Host: sharding, transposes, edge bucketing/padding, int16 index prep,
output transpose/stitch.  All numeric compute is on-device.
"""
import math
import numpy as np

# ---------------- static problem config (hardcoded per contract) -------------
N_NODES = 100000
N_EDGES = 1600000
IN_FEAT = 256
OUT_FEAT = 64
NCORES = 8
P = 128
ROWS_PER_CORE = N_NODES // NCORES              # 12500
TILES = math.ceil(ROWS_PER_CORE / P)           # 98
ROWS_PAD = TILES * P                           # 12544
POS_TOTAL = NCORES * ROWS_PAD                  # 100352 (padded position space)
BUCKET = 32768
N_BUCKETS = math.ceil(POS_TOTAL / BUCKET)      # 4
G = 8                                          # output tiles per gather batch
BATCH_SIZES = [G] * (TILES // G) + ([TILES % G] if TILES % G else [])
OOB_ROW = 300.0                                # rloc value for pad slots


def _host_prep(feat, weight, adj_rows, adj_cols, adj_vals):
    """Build per-core shards + static schedule (caps, bands, offsets)."""
    feat = np.asarray(feat, np.float32)
    weight = np.asarray(weight, np.float32)
    rows = np.asarray(adj_rows, np.int64)
    cols = np.asarray(adj_cols, np.int64)
    vals = np.asarray(adj_vals, np.float32)

    # map source node -> padded position space (core-major, 12544 per core).
    # proj is stored partition-major per core: local row l at p*TILES + t
    # where t = l // P, p = l % P (matches the phase-1 store layout).
    loc = cols % ROWS_PER_CORE
    pos = (cols // ROWS_PER_CORE) * ROWS_PAD + (loc % P) * TILES + loc // P

    # per-core contiguous edge ranges (rows sorted)
    core_edge = np.searchsorted(rows, np.arange(0, N_NODES + 1, ROWS_PER_CORE))

    # ---- pass 1: per (core, tile, bucket) edge selections; find caps -------
    sel = {}
    counts = np.zeros((NCORES, TILES, N_BUCKETS), np.int64)
    for c in range(NCORES):
        e0, e1 = core_edge[c], core_edge[c + 1]
        r = rows[e0:e1] - c * ROWS_PER_CORE
        tptr = np.searchsorted(r, np.arange(0, ROWS_PAD + 1, P))
        pc = pos[e0:e1]
        bk = pc // BUCKET
        for t in range(TILES):
            a, b = tptr[t], tptr[t + 1]
            for bb in range(N_BUCKETS):
                s = np.nonzero(bk[a:b] == bb)[0] + a + e0
                sel[(c, t, bb)] = s
                counts[c, t, bb] = len(s)

    caps = [int(np.ceil(max(1, counts[:, :, bb].max()) / P)) * P
            for bb in range(N_BUCKETS)]
    U = sum(caps) // P                         # chunks per tile
    cstart = np.cumsum([0] + caps)             # slot offset of bucket within tile

    # ---- pass 2: batch-major slot arrays -----------------------------------
    # order: for batch B: for bucket b: for tile g in batch: cap_b slots
    slots_per_tile = sum(caps)
    total_slots = TILES * slots_per_tile       # per core
    idx16 = np.zeros((NCORES, total_slots), np.int16)
    rloc = np.full((NCORES, total_slots), OOB_ROW, np.float32)
    vv = np.zeros((NCORES, total_slots), np.float32)

    # static maps (uniform across cores)
    batch_slot_base = []                       # slot offset of each batch
    off = 0
    for Gb in BATCH_SIZES:
        batch_slot_base.append(off)
        off += Gb * slots_per_tile

    # band tracking per (bucket, j) over all cores/tiles
    band_lo = [np.full(caps[bb] // P, P, np.int64) for bb in range(N_BUCKETS)]
    band_hi = [np.zeros(caps[bb] // P, np.int64) for bb in range(N_BUCKETS)]

    for c in range(NCORES):
        for B, Gb in enumerate(BATCH_SIZES):
            t0 = B * G
            boff = batch_slot_base[B]
            for bb in range(N_BUCKETS):
                for g in range(Gb):
                    t = t0 + g
                    s = sel[(c, t, bb)]
                    n = len(s)
                    o = boff + g * caps[bb]
                    idx16[c, o:o + n] = (pos[s] - bb * BUCKET).astype(np.int16)
                    rloc[c, o:o + n] = (rows[s] - (c * ROWS_PER_CORE + t * P)
                                        ).astype(np.float32)
                    vv[c, o:o + n] = vals[s]
                    # bands per j (chunk within this bucket region)
                    if n:
                        rl = rows[s] - (c * ROWS_PER_CORE + t * P)
                        for j in range(caps[bb] // P):
                            seg = rl[j * P:(j + 1) * P]
                            if len(seg):
                                band_lo[bb][j] = min(band_lo[bb][j], seg.min())
                                band_hi[bb][j] = max(band_hi[bb][j], seg.max() + 1)
                boff += Gb * caps[bb]

    # finalize bands: (lo, w); chunk (b=0, j=0) forced full-width
    bands = []
    for bb in range(N_BUCKETS):
        bl = []
        for j in range(caps[bb] // P):
            lo = int(min(band_lo[bb][j], P - 1))
            hi = int(max(band_hi[bb][j], lo + 1))
            w = hi - lo
            bl.append((lo, w))
        bands.append(bl)
    bands[0][0] = (0, P)                       # start-of-group full coverage

    # rebase rloc to band-relative; recompute per slot
    # slot -> (bucket, j) is static: within batch, bucket regions of Gb*cap_b
    for c in range(NCORES):
        for B, Gb in enumerate(BATCH_SIZES):
            boff = batch_slot_base[B]
            for bb in range(N_BUCKETS):
                cb = caps[bb]
                for g in range(Gb):
                    o = boff + g * cb
                    for j in range(cb // P):
                        lo, w = bands[bb][j]
                        sl = slice(o + j * P, o + (j + 1) * P)
                        rl = rloc[c, sl]
                        real = rl < OOB_ROW
                        rl[real] -= lo
                        rloc[c, sl] = rl
                boff += Gb * cb

    # ---- wrap idx into [16, n/16] x8 replication; pc-order rloc/vv ---------
    # gather call (B, bb): slots [call_off, call_off + Gb*cap_b)
    # within call, position i -> partition i%128, chunk i//128
    idx_sb = np.zeros((NCORES, P, total_slots // 16), np.int16)
    rl_sb = np.zeros((NCORES, P, total_slots // P), np.float32)
    vv_sb = np.zeros((NCORES, P, total_slots // P), np.float32)
    for c in range(NCORES):
        w16 = idx16[c].reshape(-1, 16).T       # [16, S/16]
        idx_sb[c] = np.tile(w16, (8, 1))
        rl_sb[c] = rloc[c].reshape(-1, P).T    # [128, S/128] (chunk cols)
        vv_sb[c] = vv[c].reshape(-1, P).T

    # ---- featT / W shards ---------------------------------------------------
    featT = np.zeros((NCORES, P, IN_FEAT // P, ROWS_PAD), np.float32)
    for c in range(NCORES):
        f = feat[c * ROWS_PER_CORE:(c + 1) * ROWS_PER_CORE]      # [12500, 256]
        ft = f.T.reshape(IN_FEAT // P, P, ROWS_PER_CORE)         # [2,128,12500]
        featT[c, :, :, :ROWS_PER_CORE] = ft.transpose(1, 0, 2)
    w_pk = weight.reshape(IN_FEAT // P, P, OUT_FEAT).transpose(1, 0, 2).copy()
    iota = np.tile(np.arange(P, dtype=np.float32), (P, 1))

    in_maps = []
    for c in range(NCORES):
        meta = np.concatenate([iota, rl_sb[c], vv_sb[c]], axis=1)
        in_maps.append({
            "featT": featT[c],
            "w": w_pk,
            "meta": meta,
            "idx": idx_sb[c],
        })
    static = dict(caps=caps, U=U, bands=bands,
                  total_slots=total_slots)
    return in_maps, static


def _build_nc(static):
    import concourse.bacc as bacc
    import concourse.bass as bass
    import concourse.mybir as mybir
    import concourse.tile as tile
    from contextlib import ExitStack

    caps, U, bands = static["caps"], static["U"], static["bands"]
    total_slots = static["total_slots"]
    KC = IN_FEAT // P                           # 2 k-chunks

    # 32KB/partition SWDGE scratch -> 2048-descriptor rings (default 1024
    # cannot hold one batched gather call)
    nc = bacc.Bacc(None, target_bir_lowering=False,
                   dynamic_dma_scratch_size=32768)
    MAXD = 1024                     # max indices per dma_gather call
                                    # (SWDGE ring capacity bound)
    featT_d = nc.dram_tensor("featT", [P, KC, ROWS_PAD], mybir.dt.float32,
                             kind="ExternalInput")
    w_d = nc.dram_tensor("w", [P, KC, OUT_FEAT], mybir.dt.float32,
                         kind="ExternalInput")
    CC = total_slots // P
    meta_d = nc.dram_tensor("meta", [P, P + 2 * CC], mybir.dt.float32,
                            kind="ExternalInput")
    idx_d = nc.dram_tensor("idx", [P, total_slots // 16], mybir.dt.int16,
                           kind="ExternalInput")
    out_d = nc.dram_tensor("out", [OUT_FEAT, ROWS_PAD], mybir.dt.float32,
                           kind="ExternalOutput")
    # proj stored partition-major: position (t*128+p) at flat row p*TILES+t
    proj_loc = nc.dram_tensor("proj_loc", [ROWS_PAD, OUT_FEAT],
                              mybir.dt.float32)
    proj_full = nc.dram_tensor("proj_full", [POS_TOTAL, OUT_FEAT],
                               mybir.dt.float32, addr_space="Shared")

    with tile.TileContext(nc) as tc, ExitStack() as ctx:
        # one Pool register per distinct gather count (pool is small)
        def call_sizes(total):
            ncalls = -(-total // MAXD)
            per = -(-total // (ncalls * P)) * P
            sizes = []
            left = total
            while left > 0:
                sizes.append(min(per, left))
                left -= sizes[-1]
            return sizes

        nidx_reg = {}
        for Gb in sorted(set(BATCH_SIZES)):
            for cb in set(caps):
                for v in call_sizes(Gb * cb):
                    if v not in nidx_reg:
                        nidx_reg[v] = nc.gpsimd.to_reg(v)
        const = ctx.enter_context(tc.tile_pool(name="const", bufs=1))

        w_t = const.tile([P, KC, OUT_FEAT], mybir.dt.float32)
        nc.sync.dma_start(w_t[:], w_d[:])
        meta_t = const.tile([P, P + 2 * CC], mybir.dt.float32)
        nc.sync.dma_start(meta_t[:], meta_d[:])
        iota_t = meta_t[:, 0:P]
        rloc_t = meta_t[:, P:P + CC]
        vv_t = meta_t[:, P + CC:P + 2 * CC]
        idx_t = const.tile([P, total_slots // 16], mybir.dt.int16)
        nc.sync.dma_start(idx_t[:], idx_d[:])

        # ---------------- phase 1: proj = feat @ W --------------------------
        with tc.tile_pool(name="p1", bufs=3) as p1, \
             tc.tile_pool(name="p1ps", bufs=2, space="PSUM") as p1ps:
            for B, Gb in enumerate(BATCH_SIZES):
                t0 = B * G
                ft = p1.tile([P, KC, Gb * P], mybir.dt.float32, tag="ft")
                nc.sync.dma_start(ft[:], featT_d[:, :, t0 * P:(t0 + Gb) * P])
                stage = p1.tile([P, Gb, OUT_FEAT], mybir.dt.float32, tag="pst")
                for g in range(Gb):
                    pp = p1ps.tile([P, OUT_FEAT], mybir.dt.float32, tag="pp")
                    for k in range(KC):
                        nc.tensor.matmul(pp[:], ft[:, k, g * P:(g + 1) * P],
                                         w_t[:, k, :],
                                         start=(k == 0), stop=(k == KC - 1))
                    nc.vector.tensor_copy(stage[:, g, :], pp[:])
                # proj_loc viewed [P, TILES, OUT]: row p*TILES+t
                pv = proj_loc.rearrange("(p t) f -> p t f", p=P)
                nc.sync.dma_start(pv[:, t0:t0 + Gb, :], stage[:])

        # ---------------- allgather ----------------------------------------
        nc.gpsimd.collective_compute(
            "AllGather", mybir.AluOpType.bypass,
            replica_groups=[list(range(NCORES))],
            ins=[proj_loc[:]], outs=[proj_full[:]],
        )

        # ---------------- phase 2: gather + segment matmul ------------------
        with tc.tile_pool(name="p2", bufs=2) as p2, \
             tc.tile_pool(name="p2i", bufs=6) as p2i, \
             tc.tile_pool(name="p2o", bufs=2) as p2o, \
             tc.tile_pool(name="p2ps", bufs=4, space="PSUM") as p2ps:
            slot_base = 0                       # running slot offset (units of slots)
            for B, Gb in enumerate(BATCH_SIZES):
                t0 = B * G
                UB = Gb * U                     # msgs chunks this batch
                msgs = p2.tile([P, UB, OUT_FEAT], mybir.dt.float32, tag="msgs")
                # gather calls per bucket
                coff = 0
                for bb in range(N_BUCKETS):
                    cb = caps[bb]
                    base = bb * BUCKET
                    nrows = min(BUCKET, POS_TOTAL - base)
                    for nidx in call_sizes(Gb * cb):
                        i0 = slot_base + coff * P
                        nc.gpsimd.dma_gather(
                            msgs[:, coff:coff + nidx // P, :],
                            proj_full[base:base + nrows, :],
                            idx_t[:, i0 // 16:(i0 + nidx) // 16],
                            nidx, nidx_reg[nidx], OUT_FEAT,
                        )
                        coff += nidx // P
                # per tile: U chunk matmuls into psumT
                out_stage = p2o.tile([OUT_FEAT, Gb * P], mybir.dt.float32,
                                     tag="ost")
                for g in range(Gb):
                    pt = p2ps.tile([OUT_FEAT, P], mybir.dt.float32, tag="pt")
                    c_local = 0
                    bchunk = 0                  # batch chunk base for bucket
                    for bb in range(N_BUCKETS):
                        cbP = caps[bb] // P
                        for j in range(cbP):
                            mc = bchunk + g * cbP + j       # msgs chunk index
                            col = slot_base // P + mc       # rloc/vv column
                            lo, w = bands[bb][j]
                            ind = p2i.tile([P, P], mybir.dt.float32, tag="ind")
                            nc.vector.tensor_scalar(
                                ind[:, :w], iota_t[:, :w],
                                rloc_t[:, col:col + 1],
                                vv_t[:, col:col + 1],
                                mybir.AluOpType.is_equal,
                                mybir.AluOpType.mult,
                            )
                            nc.tensor.matmul(
                                pt[:, lo:lo + w],
                                msgs[:, mc, :], ind[:, :w],
                                start=(c_local == 0), stop=(c_local == U - 1),
                            )
                            c_local += 1
                        bchunk += Gb * cbP
                    nc.vector.tensor_copy(out_stage[:, g * P:(g + 1) * P], pt[:])
                nc.sync.dma_start(out_d[:, t0 * P:(t0 + Gb) * P], out_stage[:])
                slot_base += Gb * sum(caps)

    # Bacc lowering: wait-splitting (HW allows 1 wait/inst), matmul-wait
    # redistribution, library reload insertion, extended-inst ISA codegen.
    nc.compile()
    return nc


LAST_RESULTS = None


def kernel(feat, weight, adj_rows, adj_cols, adj_vals):
    global LAST_RESULTS
    from concourse.bass_utils import run_bass_kernel_spmd

    in_maps, static = _host_prep(feat, weight, adj_rows, adj_cols, adj_vals)
    nc = _build_nc(static)
    res = run_bass_kernel_spmd(nc, in_maps, core_ids=list(range(NCORES)))
    LAST_RESULTS = res
    out = np.concatenate(
        [res.results[c]["out"][:, :ROWS_PER_CORE].T for c in range(NCORES)],
        axis=0)
    return np.ascontiguousarray(out)


if __name__ == "__main__":
    # smoke-test host prep only
    rng = np.random.default_rng(0)
    feat = rng.standard_normal((N_NODES, IN_FEAT), np.float32)
    weight = rng.standard_normal((IN_FEAT, OUT_FEAT), np.float32)
    rows = np.sort(rng.integers(0, N_NODES, N_EDGES))
    cols = rng.integers(0, N_NODES, N_EDGES)
    vals = rng.random(N_EDGES, np.float32)
    in_maps, static = _host_prep(feat, weight, rows, cols, vals)
    print("caps", static["caps"], "U", static["U"])
    print("bands[0][:4]", static["bands"][0][:4])
    print("bands[1][:4]", static["bands"][1][:4])



# revision 3
# speedup vs baseline: 1.4609x; 1.4609x over previous
"""GNN message passing on 8 Trainium2 NeuronCores.

out = segment_sum(adj_vals * (feat @ W)[adj_cols], adj_rows)
N=100000 nodes, E=1600000 edges, 256 -> 64 features, fp32.

Sharding: nodes (rows of feat / output) across 8 cores; weight replicated;
edges partitioned by destination row (adj_rows sorted -> contiguous); each
core's scatter-add is local, with an AllGather of projected source features.

Device pipeline per core:
  phase 1: proj_local = featT_slice.T @ W       (PE, PSUM-accumulated)
           AllGather(proj_local) -> proj_full   (internal DRAM, Shared)
  phase 2: per batch of G output tiles:
             dma_gather (4 int16-bucket calls) proj rows -> msgs SBUF
             per 128-edge chunk: indicator[128,W] = (iota==rloc)*val  (DVE)
             psumT[64,128] += msgs_chunk.T @ indicator                (PE)
           drain psum -> out[64, :] -> DRAM
Host: sharding, transposes, edge bucketing/padding, int16 index prep,
output transpose/stitch.  All numeric compute is on-device.
"""
import math
import numpy as np

# ---------------- static problem config (hardcoded per contract) -------------
N_NODES = 100000
N_EDGES = 1600000
IN_FEAT = 256
OUT_FEAT = 64
NCORES = 8
P = 128
ROWS_PER_CORE = N_NODES // NCORES              # 12500
TILES = math.ceil(ROWS_PER_CORE / P)           # 98
ROWS_PAD = TILES * P                           # 12544
POS_TOTAL = NCORES * ROWS_PAD                  # 100352 (padded position space)
BUCKET = 32768
N_BUCKETS = math.ceil(POS_TOTAL / BUCKET)      # 4
G = 8                                          # output tiles per gather batch
BATCH_SIZES = [G] * (TILES // G) + ([TILES % G] if TILES % G else [])
OOB_ROW = 300.0                                # rloc value for pad slots


def _host_prep(feat, weight, adj_rows, adj_cols, adj_vals):
    """Build per-core shards + static schedule (caps, bands, offsets)."""
    feat = np.asarray(feat, np.float32)
    weight = np.asarray(weight, np.float32)
    rows = np.asarray(adj_rows, np.int64)
    cols = np.asarray(adj_cols, np.int64)
    vals = np.asarray(adj_vals, np.float32)

    # map source node -> padded position space (core-major, 12544 per core).
    # proj is stored partition-major per core: local row l at p*TILES + t
    # where t = l // P, p = l % P (matches the phase-1 store layout).
    loc = cols % ROWS_PER_CORE
    pos = (cols // ROWS_PER_CORE) * ROWS_PAD + (loc % P) * TILES + loc // P

    # per-core contiguous edge ranges (rows sorted)
    core_edge = np.searchsorted(rows, np.arange(0, N_NODES + 1, ROWS_PER_CORE))

    # ---- pass 1: per (core, tile, bucket) edge selections; find caps -------
    sel = {}
    counts = np.zeros((NCORES, TILES, N_BUCKETS), np.int64)
    for c in range(NCORES):
        e0, e1 = core_edge[c], core_edge[c + 1]
        r = rows[e0:e1] - c * ROWS_PER_CORE
        tptr = np.searchsorted(r, np.arange(0, ROWS_PAD + 1, P))
        pc = pos[e0:e1]
        bk = pc // BUCKET
        for t in range(TILES):
            a, b = tptr[t], tptr[t + 1]
            for bb in range(N_BUCKETS):
                s = np.nonzero(bk[a:b] == bb)[0] + a + e0
                sel[(c, t, bb)] = s
                counts[c, t, bb] = len(s)

    caps = [int(np.ceil(max(1, counts[:, :, bb].max()) / P)) * P
            for bb in range(N_BUCKETS)]
    U = sum(caps) // P                         # chunks per tile
    cstart = np.cumsum([0] + caps)             # slot offset of bucket within tile

    # ---- pass 2: batch-major slot arrays -----------------------------------
    # order: for batch B: for bucket b: for tile g in batch: cap_b slots
    slots_per_tile = sum(caps)
    total_slots = TILES * slots_per_tile       # per core
    idx16 = np.zeros((NCORES, total_slots), np.int16)
    rloc = np.full((NCORES, total_slots), OOB_ROW, np.float32)
    vv = np.zeros((NCORES, total_slots), np.float32)

    # static maps (uniform across cores)
    batch_slot_base = []                       # slot offset of each batch
    off = 0
    for Gb in BATCH_SIZES:
        batch_slot_base.append(off)
        off += Gb * slots_per_tile

    # band tracking per (bucket, j) over all cores/tiles
    band_lo = [np.full(caps[bb] // P, P, np.int64) for bb in range(N_BUCKETS)]
    band_hi = [np.zeros(caps[bb] // P, np.int64) for bb in range(N_BUCKETS)]

    for c in range(NCORES):
        for B, Gb in enumerate(BATCH_SIZES):
            t0 = B * G
            boff = batch_slot_base[B]
            for bb in range(N_BUCKETS):
                for g in range(Gb):
                    t = t0 + g
                    s = sel[(c, t, bb)]
                    n = len(s)
                    o = boff + g * caps[bb]
                    idx16[c, o:o + n] = (pos[s] - bb * BUCKET).astype(np.int16)
                    rloc[c, o:o + n] = (rows[s] - (c * ROWS_PER_CORE + t * P)
                                        ).astype(np.float32)
                    vv[c, o:o + n] = vals[s]
                    # bands per j (chunk within this bucket region)
                    if n:
                        rl = rows[s] - (c * ROWS_PER_CORE + t * P)
                        for j in range(caps[bb] // P):
                            seg = rl[j * P:(j + 1) * P]
                            if len(seg):
                                band_lo[bb][j] = min(band_lo[bb][j], seg.min())
                                band_hi[bb][j] = max(band_hi[bb][j], seg.max() + 1)
                boff += Gb * caps[bb]

    # finalize bands: (lo, w); chunk (b=0, j=0) forced full-width
    bands = []
    for bb in range(N_BUCKETS):
        bl = []
        for j in range(caps[bb] // P):
            lo = int(min(band_lo[bb][j], P - 1))
            hi = int(max(band_hi[bb][j], lo + 1))
            w = hi - lo
            bl.append((lo, w))
        bands.append(bl)
    bands[0][0] = (0, P)                       # start-of-group full coverage

    # rebase rloc to band-relative; recompute per slot
    # slot -> (bucket, j) is static: within batch, bucket regions of Gb*cap_b
    for c in range(NCORES):
        for B, Gb in enumerate(BATCH_SIZES):
            boff = batch_slot_base[B]
            for bb in range(N_BUCKETS):
                cb = caps[bb]
                for g in range(Gb):
                    o = boff + g * cb
                    for j in range(cb // P):
                        lo, w = bands[bb][j]
                        sl = slice(o + j * P, o + (j + 1) * P)
                        rl = rloc[c, sl]
                        real = rl < OOB_ROW
                        rl[real] -= lo
                        rloc[c, sl] = rl
                boff += Gb * cb

    # ---- wrap idx into [16, n/16] x8 replication; pc-order rloc/vv ---------
    # gather call (B, bb): slots [call_off, call_off + Gb*cap_b)
    # within call, position i -> partition i%128, chunk i//128
    idx_sb = np.zeros((NCORES, P, total_slots // 16), np.int16)
    rl_sb = np.zeros((NCORES, P, total_slots // P), np.float32)
    vv_sb = np.zeros((NCORES, P, total_slots // P), np.float32)
    for c in range(NCORES):
        w16 = idx16[c].reshape(-1, 16).T       # [16, S/16]
        idx_sb[c] = np.tile(w16, (8, 1))
        rl_sb[c] = rloc[c].reshape(-1, P).T    # [128, S/128] (chunk cols)
        vv_sb[c] = vv[c].reshape(-1, P).T

    # ---- featT / W shards ---------------------------------------------------
    featT = np.zeros((NCORES, P, IN_FEAT // P, ROWS_PAD), np.float32)
    for c in range(NCORES):
        f = feat[c * ROWS_PER_CORE:(c + 1) * ROWS_PER_CORE]      # [12500, 256]
        ft = f.T.reshape(IN_FEAT // P, P, ROWS_PER_CORE)         # [2,128,12500]
        featT[c, :, :, :ROWS_PER_CORE] = ft.transpose(1, 0, 2)
    w_pk = weight.reshape(IN_FEAT // P, P, OUT_FEAT).transpose(1, 0, 2).copy()
    iota = np.tile(np.arange(P, dtype=np.float32), (P, 1))

    in_maps = []
    for c in range(NCORES):
        meta = np.concatenate([iota, rl_sb[c], vv_sb[c]], axis=1)
        in_maps.append({
            "featT": featT[c],
            "w": w_pk,
            "meta": meta,
            "idx": idx_sb[c],
        })
    static = dict(caps=caps, U=U, bands=bands,
                  total_slots=total_slots)
    return in_maps, static


def _build_nc(static):
    import concourse.bacc as bacc
    import concourse.bass as bass
    import concourse.mybir as mybir
    import concourse.tile as tile
    from contextlib import ExitStack

    caps, U, bands = static["caps"], static["U"], static["bands"]
    total_slots = static["total_slots"]
    KC = IN_FEAT // P                           # 2 k-chunks

    # 64KB/partition SWDGE scratch -> two 2048-descriptor rings. Two SWDGE
    # queues let one call's descriptor generation overlap the other's
    # ring-drain/transfer wait (measured: DGE ~2.7us of the 8.6us/call; the
    # rest is ring-space wait, which single-queue serializes).
    nc = bacc.Bacc(None, target_bir_lowering=False,
                   dynamic_dma_scratch_size=65536, num_swdge_queues=2)
    MAXD = 1024                     # max indices per dma_gather call
                                    # (SWDGE ring capacity bound)
    featT_d = nc.dram_tensor("featT", [P, KC, ROWS_PAD], mybir.dt.float32,
                             kind="ExternalInput")
    w_d = nc.dram_tensor("w", [P, KC, OUT_FEAT], mybir.dt.float32,
                         kind="ExternalInput")
    CC = total_slots // P
    meta_d = nc.dram_tensor("meta", [P, P + 2 * CC], mybir.dt.float32,
                            kind="ExternalInput")
    idx_d = nc.dram_tensor("idx", [P, total_slots // 16], mybir.dt.int16,
                           kind="ExternalInput")
    out_d = nc.dram_tensor("out", [OUT_FEAT, ROWS_PAD], mybir.dt.float32,
                           kind="ExternalOutput")
    # proj stored partition-major: position (t*128+p) at flat row p*TILES+t
    proj_loc = nc.dram_tensor("proj_loc", [ROWS_PAD, OUT_FEAT],
                              mybir.dt.float32)
    proj_full = nc.dram_tensor("proj_full", [POS_TOTAL, OUT_FEAT],
                               mybir.dt.float32, addr_space="Shared")

    with tile.TileContext(nc) as tc, ExitStack() as ctx:
        # one Pool register per distinct gather count (pool is small)
        def call_sizes(total):
            ncalls = -(-total // MAXD)
            per = -(-total // (ncalls * P)) * P
            sizes = []
            left = total
            while left > 0:
                sizes.append(min(per, left))
                left -= sizes[-1]
            return sizes

        nidx_reg = {}
        for Gb in sorted(set(BATCH_SIZES)):
            for cb in set(caps):
                for v in call_sizes(Gb * cb):
                    if v not in nidx_reg:
                        nidx_reg[v] = nc.gpsimd.to_reg(v)
        const = ctx.enter_context(tc.tile_pool(name="const", bufs=1))

        w_t = const.tile([P, KC, OUT_FEAT], mybir.dt.float32)
        nc.sync.dma_start(w_t[:], w_d[:])
        meta_t = const.tile([P, P + 2 * CC], mybir.dt.float32)
        nc.sync.dma_start(meta_t[:], meta_d[:])
        iota_t = meta_t[:, 0:P]
        rloc_t = meta_t[:, P:P + CC]
        vv_t = meta_t[:, P + CC:P + 2 * CC]
        idx_t = const.tile([P, total_slots // 16], mybir.dt.int16)
        nc.sync.dma_start(idx_t[:], idx_d[:])

        # ---------------- phase 1: proj = feat @ W --------------------------
        with tc.tile_pool(name="p1", bufs=3) as p1, \
             tc.tile_pool(name="p1ps", bufs=2, space="PSUM") as p1ps:
            for B, Gb in enumerate(BATCH_SIZES):
                t0 = B * G
                ft = p1.tile([P, KC, Gb * P], mybir.dt.float32, tag="ft")
                nc.sync.dma_start(ft[:], featT_d[:, :, t0 * P:(t0 + Gb) * P])
                stage = p1.tile([P, Gb, OUT_FEAT], mybir.dt.float32, tag="pst")
                for g in range(Gb):
                    pp = p1ps.tile([P, OUT_FEAT], mybir.dt.float32, tag="pp")
                    for k in range(KC):
                        nc.tensor.matmul(pp[:], ft[:, k, g * P:(g + 1) * P],
                                         w_t[:, k, :],
                                         start=(k == 0), stop=(k == KC - 1))
                    nc.vector.tensor_copy(stage[:, g, :], pp[:])
                # proj_loc viewed [P, TILES, OUT]: row p*TILES+t
                pv = proj_loc.rearrange("(p t) f -> p t f", p=P)
                nc.sync.dma_start(pv[:, t0:t0 + Gb, :], stage[:])

        # ---------------- allgather ----------------------------------------
        nc.gpsimd.collective_compute(
            "AllGather", mybir.AluOpType.bypass,
            replica_groups=[list(range(NCORES))],
            ins=[proj_loc[:]], outs=[proj_full[:]],
        )

        # ---------------- phase 2: gather + segment matmul ------------------
        with tc.tile_pool(name="p2", bufs=2) as p2, \
             tc.tile_pool(name="p2i", bufs=6) as p2i, \
             tc.tile_pool(name="p2o", bufs=2) as p2o, \
             tc.tile_pool(name="p2ps", bufs=4, space="PSUM") as p2ps:
            slot_base = 0                       # running slot offset (units of slots)
            qc = 0                              # gather call counter (ring select)
            for B, Gb in enumerate(BATCH_SIZES):
                t0 = B * G
                UB = Gb * U                     # msgs chunks this batch
                msgs = p2.tile([P, UB, OUT_FEAT], mybir.dt.float32, tag="msgs")
                # gather calls per bucket
                coff = 0
                for bb in range(N_BUCKETS):
                    cb = caps[bb]
                    base = bb * BUCKET
                    nrows = min(BUCKET, POS_TOTAL - base)
                    for nidx in call_sizes(Gb * cb):
                        i0 = slot_base + coff * P
                        nc.gpsimd.dma_gather(
                            msgs[:, coff:coff + nidx // P, :],
                            proj_full[base:base + nrows, :],
                            idx_t[:, i0 // 16:(i0 + nidx) // 16],
                            nidx, nidx_reg[nidx], OUT_FEAT,
                            queue_num=qc % 2,
                        )
                        qc += 1
                        coff += nidx // P
                # per tile: U chunk matmuls into psumT
                out_stage = p2o.tile([OUT_FEAT, Gb * P], mybir.dt.float32,
                                     tag="ost")
                for g in range(Gb):
                    pt = p2ps.tile([OUT_FEAT, P], mybir.dt.float32, tag="pt")
                    c_local = 0
                    bchunk = 0                  # batch chunk base for bucket
                    for bb in range(N_BUCKETS):
                        cbP = caps[bb] // P
                        for j in range(cbP):
                            mc = bchunk + g * cbP + j       # msgs chunk index
                            col = slot_base // P + mc       # rloc/vv column
                            lo, w = bands[bb][j]
                            ind = p2i.tile([P, P], mybir.dt.float32, tag="ind")
                            nc.vector.tensor_scalar(
                                ind[:, :w], iota_t[:, :w],
                                rloc_t[:, col:col + 1],
                                vv_t[:, col:col + 1],
                                mybir.AluOpType.is_equal,
                                mybir.AluOpType.mult,
                            )
                            nc.tensor.matmul(
                                pt[:, lo:lo + w],
                                msgs[:, mc, :], ind[:, :w],
                                start=(c_local == 0), stop=(c_local == U - 1),
                            )
                            c_local += 1
                        bchunk += Gb * cbP
                    nc.vector.tensor_copy(out_stage[:, g * P:(g + 1) * P], pt[:])
                nc.sync.dma_start(out_d[:, t0 * P:(t0 + Gb) * P], out_stage[:])
                slot_base += Gb * sum(caps)

    # Bacc lowering: wait-splitting (HW allows 1 wait/inst), matmul-wait
    # redistribution, library reload insertion, extended-inst ISA codegen.
    nc.compile()
    return nc


LAST_RESULTS = None


def kernel(feat, weight, adj_rows, adj_cols, adj_vals):
    global LAST_RESULTS
    from concourse.bass_utils import run_bass_kernel_spmd

    in_maps, static = _host_prep(feat, weight, adj_rows, adj_cols, adj_vals)
    nc = _build_nc(static)
    res = run_bass_kernel_spmd(nc, in_maps, core_ids=list(range(NCORES)))
    LAST_RESULTS = res
    out = np.concatenate(
        [res.results[c]["out"][:, :ROWS_PER_CORE].T for c in range(NCORES)],
        axis=0)
    return np.ascontiguousarray(out)


if __name__ == "__main__":
    # smoke-test host prep only
    rng = np.random.default_rng(0)
    feat = rng.standard_normal((N_NODES, IN_FEAT), np.float32)
    weight = rng.standard_normal((IN_FEAT, OUT_FEAT), np.float32)
    rows = np.sort(rng.integers(0, N_NODES, N_EDGES))
    cols = rng.integers(0, N_NODES, N_EDGES)
    vals = rng.random(N_EDGES, np.float32)
    in_maps, static = _host_prep(feat, weight, rows, cols, vals)
    print("caps", static["caps"], "U", static["U"])
    print("bands[0][:4]", static["bands"][0][:4])
    print("bands[1][:4]", static["bands"][1][:4])

